# revision 23
# baseline (speedup 1.0000x reference)
"""ProDualNet GNN message-passing layer on 8 Trainium2 NeuronCores.

Sharding: data-parallel over B (4 samples) x 2-way sharding of the N=2048
residue axis -> 8 cores, 1024 nodes each. Node-level tables (hV, frames Q,
atom rows) are computed redundantly per sample pair; per-edge and per-self
accesses are served from per-core DRAM tables via the ANT dma_gather ucode
op (transpose mode feeds the PE channel-on-partition directly). All
addressing is index-driven, so one SPMD module serves all 8 cores.
"""
import sys, os
for _p in ('/opt/trn_rl_repo', '/root/.axon_site/_ro/trn_rl_repo'):
    if os.path.isdir(_p) and _p not in sys.path:
        sys.path.insert(0, _p)

import numpy as np
from contextlib import ExitStack

import concourse.bass as bass
import concourse.bacc as bacc
import concourse.mybir as mybir
import concourse.tile as tile
from concourse.bass_utils import run_bass_kernel_spmd
from concourse.masks import make_identity

F32 = mybir.dt.float32
BF16 = mybir.dt.bfloat16
AF = mybir.ActivationFunctionType
OP = mybir.AluOpType
AX = mybir.AxisListType

B, N, K, H = 4, 2048, 30, 128
NODE_IN, EDGE_IN, FF = 21, 16, 512
GROW = 64            # geometry table row length (256B)
NPC = N // 2         # nodes per core


def ap_of(t, off, dims, p0=0, pcnt=None):
    base = t[:]
    pstep, pc = base.ap[0]
    if pcnt is None:
        pcnt = pc - p0
    return bass.AP(tensor=base.tensor, offset=base.offset + off + p0 * pstep,
                   ap=[[pstep, pcnt]] + [list(d) for d in dims])


def dap(t, off, dims):
    base = t.ap()
    return bass.AP(tensor=base.tensor, offset=off, ap=[list(d) for d in dims])


def build_kernel(n_sample, node_cnt, num_devices):
    MP = n_sample // 128
    CSP = 3 * MP + 4
    BLK = node_cnt // 128
    ICOL = BLK * 248      # idx16r columns per block: 3840 edge + 128 self idx
    nc = bacc.Bacc("TRN2", num_devices=num_devices)

    xp = nc.dram_tensor("xp", [3 * n_sample + 4, 3], F32, kind="ExternalInput")
    xatoms = nc.dram_tensor("xatoms", [n_sample, 12], F32, kind="ExternalInput")
    idx16r = nc.dram_tensor("idx16r", [128, ICOL], mybir.dt.int16, kind="ExternalInput")
    WIN = {}
    for nm, shp in [("wv_w", [NODE_IN, H]), ("wv_b", [H]), ("we_w", [EDGE_IN, H]),
                    ("we_b", [H]), ("w1", [3 * H, H]), ("w1_b", [H]),
                    ("w2", [H, H]), ("w2_b", [H]), ("w3", [H, H]), ("w3_b", [H]),
                    ("wi", [H, FF]), ("wi_b", [FF]), ("wo", [FF, H]), ("wo_b", [H]),
                    ("ln1_g", [H]), ("ln1_b", [H]), ("ln2_g", [H]), ("ln2_b", [H])]:
        WIN[nm] = nc.dram_tensor(nm, shp, F32, kind="ExternalInput")
    out = nc.dram_tensor("out", [node_cnt, H], F32, kind="ExternalOutput")
    hv_table = nc.dram_tensor("hv_table", [n_sample, H], BF16, kind="Internal")
    hvf_table = nc.dram_tensor("hvf_table", [n_sample, H], F32, kind="Internal")
    geom_table = nc.dram_tensor("geom_table", [n_sample, GROW], F32, kind="Internal")

    with ExitStack() as ctx:
        tc = ctx.enter_context(tile.TileContext(nc))
        cons = ctx.enter_context(tc.tile_pool(name="cons", bufs=1))
        ach = ctx.enter_context(tc.tile_pool(name="ach", bufs=1))
        gp = ctx.enter_context(tc.tile_pool(name="gp", bufs=3))
        hvn = ctx.enter_context(tc.tile_pool(name="hvn", bufs=3))
        eqp = ctx.enter_context(tc.tile_pool(name="eqp", bufs=2))
        m1p = ctx.enter_context(tc.tile_pool(name="m1p", bufs=3))
        m2p = ctx.enter_context(tc.tile_pool(name="m2p", bufs=3))
        trk = ctx.enter_context(tc.tile_pool(name="trk", bufs=2))
        ps5 = ctx.enter_context(tc.tile_pool(name="ps5", bufs=2, space="PSUM"))
        psD = ctx.enter_context(tc.tile_pool(name="psD", bufs=1, space="PSUM"))
        psS = ctx.enter_context(tc.tile_pool(name="psS", bufs=2, space="PSUM"))

        v, sc, pe, gs = nc.vector, nc.scalar, nc.tensor, nc.gpsimd

        def pss():
            return psS.tile([128, 128], F32, tag="pss", name="pss")

        ident = cons.tile([128, 128], F32)
        make_identity(nc, ident[:])
        identb = cons.tile([128, 128], BF16)
        v.tensor_copy(out=identb[:], in_=ident[:])
        c_one = cons.tile([128, 1], F32)
        v.memset(c_one[:], 1.0)
        c_eps = cons.tile([128, 1], F32)
        v.memset(c_eps[:], 1e-5)

        def load_w(dram, p, f, dt=F32):
            t = cons.tile([p, f], dt, name="w_" + dram.name, tag="w_" + dram.name)
            nc.sync.dma_start(out=t[:], in_=dram.ap())
            return t
        wv_sb = load_w(WIN["wv_w"], NODE_IN, H)
        we_sb = load_w(WIN["we_w"], EDGE_IN, H)
        w1a_sb = cons.tile([H, H], F32)
        nc.sync.dma_start(out=w1a_sb[:], in_=dap(WIN["w1"], 0, [[H, H], [1, H]]))
        w1b_sb = cons.tile([H, H], F32)
        nc.sync.dma_start(out=w1b_sb[:], in_=dap(WIN["w1"], H * H, [[H, H], [1, H]]))
        w1c_sb = cons.tile([H, H], F32)
        nc.sync.dma_start(out=w1c_sb[:], in_=dap(WIN["w1"], 2 * H * H, [[H, H], [1, H]]))
        w1cb = cons.tile([H, H], BF16)
        v.tensor_copy(out=w1cb[:], in_=w1c_sb[:])
        w2_sb = load_w(WIN["w2"], H, H)
        w2b = cons.tile([H, H], BF16)
        v.tensor_copy(out=w2b[:], in_=w2_sb[:])
        w3_sb = load_w(WIN["w3"], H, H)
        w3p = cons.tile([H, H], BF16)
        sc.mul(out=w3p[:], in_=w3_sb[:], mul=1.0 / K)
        wi_sb = load_w(WIN["wi"], H, FF)
        wo_b16 = cons.tile([128, 4, H], BF16)
        gs0 = nc.gpsimd
        gs0.dma_start(out=wo_b16[:], in_=dap(WIN["wo"], 0,
                                             [[H, 128], [128 * H, 4], [1, H]]))

        def load_vec(dram, n=H):
            t = cons.tile([n, 1], F32, name="v_" + dram.name, tag="v_" + dram.name)
            nc.sync.dma_start(out=t[:], in_=dap(dram, 0, [[1, n], [0, 1]]))
            return t
        wvb_t = load_vec(WIN["wv_b"])
        w2b_t = load_vec(WIN["w2_b"])
        wob_t = load_vec(WIN["wo_b"])
        web_col = load_vec(WIN["we_b"])
        w1b_col = load_vec(WIN["w1_b"])
        wib_t = cons.tile([128, 4], F32)
        nc.sync.dma_start(out=wib_t[:], in_=dap(WIN["wi_b"], 0, [[1, 128], [128, 4]]))

        # broadcast rows (ln affine + w3 bias) via PE outer product
        ones1 = cons.tile([1, 128], F32)
        v.memset(ones1[:], 1.0)
        lnrow = cons.tile([1, 5 * 128], F32)
        for i, nm in enumerate(("ln1_g", "ln1_b", "ln2_g", "ln2_b", "w3_b")):
            nc.sync.dma_start(out=lnrow[:, 128 * i:128 * (i + 1)],
                              in_=dap(WIN[nm], 0, [[0, 1], [1, H]]))
        lnbc = cons.tile([128, 5, 128], F32)
        for i in range(5):
            pb = pss()
            pe.matmul(out=pb[:], lhsT=ones1[:], rhs=lnrow[:, 128 * i:128 * (i + 1)],
                      start=True, stop=True)
            v.tensor_copy(out=lnbc[:, i, :], in_=pb[:])

        # fused edge weights wbf = we_w @ w1b replicated at rows 32j (zero-padded)
        pt = pss()
        pe.matmul(out=pt[:, 0:EDGE_IN], lhsT=we_sb[:], rhs=ident[0:EDGE_IN, 0:EDGE_IN],
                  is_transpose=True, start=True, stop=True)
        wewT = cons.tile([H, EDGE_IN], F32)
        v.tensor_copy(out=wewT[:], in_=pt[:, 0:EDGE_IN])
        pw = psS.tile([EDGE_IN, 128], F32, tag="pss")
        pe.matmul(out=pw[:], lhsT=wewT[:], rhs=w1b_sb[:], start=True, stop=True)
        wbf128 = cons.tile([128, H], F32)
        v.memset(wbf128[:], 0.0)
        for j in range(4):
            v.tensor_copy(out=wbf128[32 * j:32 * j + EDGE_IN, :], in_=pw[:])

        # bias1 = we_b @ w1b + w1_b
        pb1 = psS.tile([H, 1], F32, tag="pss")
        pe.matmul(out=pb1[:], lhsT=w1b_sb[:], rhs=web_col[:], start=True, stop=True)
        bias1 = cons.tile([H, 1], F32)
        v.tensor_add(out=bias1[:], in0=pb1[:], in1=w1b_col[:])

        # ---------- phase A: chain geometry ----------
        CX = ach.tile([128, CSP, 3], F32)
        nc.sync.dma_start(out=CX[:], in_=dap(xp, 0, [[9 * MP, 128], [1, 3 * CSP]]))
        nT = CSP - 1
        dX = ach.tile([128, nT, 3], F32)
        v.tensor_sub(out=dX[:], in0=ap_of(CX, 3, [[3, nT], [1, 3]]),
                     in1=ap_of(CX, 0, [[3, nT], [1, 3]]))
        sq = ach.tile([128, nT, 3], F32)
        v.tensor_mul(out=sq[:], in0=dX[:], in1=dX[:])
        ssu = ach.tile([128, nT], F32)
        v.tensor_reduce(out=ssu[:], in_=sq[:], axis=AX.X, op=OP.add)
        v.tensor_scalar_max(out=ssu[:], in0=ssu[:], scalar1=1e-24)
        sc.activation(out=ssu[:], in_=ssu[:], func=AF.Sqrt)
        rinv = ach.tile([128, nT], F32)
        v.reciprocal(out=rinv[:], in_=ssu[:])
        Ud = ach.tile([128, nT, 6], F32)
        v.tensor_tensor(out=ap_of(Ud, 0, [[6, nT], [1, 3]]), in0=dX[:],
                        in1=ap_of(rinv, 0, [[1, nT], [0, 3]]), op=OP.mult)
        v.tensor_copy(out=ap_of(Ud, 3, [[6, nT], [1, 3]]),
                      in_=ap_of(Ud, 0, [[6, nT], [1, 3]]))

        nC = nT - 1
        P1 = ach.tile([128, nC, 3], F32)
        v.tensor_tensor(out=P1[:], in0=ap_of(Ud, 1, [[6, nC], [1, 3]]),
                        in1=ap_of(Ud, 8, [[6, nC], [1, 3]]), op=OP.mult)
        P2 = ach.tile([128, nC, 3], F32)
        v.tensor_tensor(out=P2[:], in0=ap_of(Ud, 2, [[6, nC], [1, 3]]),
                        in1=ap_of(Ud, 7, [[6, nC], [1, 3]]), op=OP.mult)
        C01 = ach.tile([128, nC, 3], F32)
        v.tensor_sub(out=C01[:], in0=P1[:], in1=P2[:])
        v.tensor_mul(out=sq[:, 0:nC, :], in0=C01[:], in1=C01[:])
        ssn = ach.tile([128, nC], F32)
        v.tensor_reduce(out=ssn[:], in_=sq[:, 0:nC, :], axis=AX.X, op=OP.add)
        v.tensor_scalar_max(out=ssn[:], in0=ssn[:], scalar1=1e-24)
        sc.activation(out=ssn[:], in_=ssn[:], func=AF.Sqrt)
        rin2 = ach.tile([128, nC], F32)
        v.reciprocal(out=rin2[:], in_=ssn[:])
        n0d = ach.tile([128, nC, 6], F32)
        v.tensor_tensor(out=ap_of(n0d, 0, [[6, nC], [1, 3]]), in0=C01[:],
                        in1=ap_of(rin2, 0, [[1, nC], [0, 3]]), op=OP.mult)
        v.tensor_copy(out=ap_of(n0d, 3, [[6, nC], [1, 3]]),
                      in_=ap_of(n0d, 0, [[6, nC], [1, 3]]))

        nA = nC - 1
        cosA = ach.tile([128, nA], F32)
        tmp3 = ach.tile([128, nA, 3], F32)
        v.tensor_tensor(out=tmp3[:], in0=ap_of(Ud, 0, [[6, nA], [1, 3]]),
                        in1=ap_of(Ud, 6, [[6, nA], [1, 3]]), op=OP.mult)
        v.tensor_reduce(out=cosA[:], in_=tmp3[:], axis=AX.X, op=OP.add)
        cosD = ach.tile([128, nA], F32)
        v.tensor_tensor(out=tmp3[:], in0=ap_of(n0d, 0, [[6, nA], [1, 3]]),
                        in1=ap_of(n0d, 6, [[6, nA], [1, 3]]), op=OP.mult)
        v.tensor_reduce(out=cosD[:], in_=tmp3[:], axis=AX.X, op=OP.add)
        v.tensor_tensor(out=tmp3[:], in0=ap_of(n0d, 1, [[6, nA], [1, 3]]),
                        in1=ap_of(n0d, 8, [[6, nA], [1, 3]]), op=OP.mult)
        P1a = ach.tile([128, nA, 3], F32)
        v.tensor_copy(out=P1a[:], in_=tmp3[:])
        v.tensor_tensor(out=tmp3[:], in0=ap_of(n0d, 2, [[6, nA], [1, 3]]),
                        in1=ap_of(n0d, 7, [[6, nA], [1, 3]]), op=OP.mult)
        crn = ach.tile([128, nA, 3], F32)
        v.tensor_sub(out=crn[:], in0=P1a[:], in1=tmp3[:])
        v.tensor_tensor(out=tmp3[:], in0=crn[:], in1=ap_of(Ud, 6, [[6, nA], [1, 3]]),
                        op=OP.mult)
        sdot = ach.tile([128, nA], F32)
        v.tensor_reduce(out=sdot[:], in_=tmp3[:], axis=AX.X, op=OP.add, negate=True)
        sgD = ach.tile([128, nA], F32)
        sc.activation(out=sgD[:], in_=sdot[:], func=AF.Sign)

        for c_ in (cosD, cosA):
            v.tensor_scalar_min(out=c_[:], in0=c_[:], scalar1=1.0 - 1e-7)
            v.tensor_scalar_max(out=c_[:], in0=c_[:], scalar1=-1.0 + 1e-7)
        sinD = ach.tile([128, nA], F32)
        sc.activation(out=sinD[:], in_=cosD[:], func=AF.Square)
        sc.activation(out=sinD[:], in_=sinD[:], func=AF.Sqrt, scale=-1.0, bias=c_one[:])
        v.tensor_mul(out=sinD[:], in0=sinD[:], in1=sgD[:])
        sinA = ach.tile([128, nA], F32)
        sc.activation(out=sinA[:], in_=cosA[:], func=AF.Square)
        sc.activation(out=sinA[:], in_=sinA[:], func=AF.Sqrt, scale=-1.0, bias=c_one[:])

        CT = ach.tile([128, MP, 32], F32)
        v.memset(CT[:], 0.0)
        for src, col in ((cosD, 0), (sinD, 3), (cosA, 6), (sinA, 9)):
            v.tensor_copy(out=ap_of(CT, col, [[32, MP], [1, 3]]),
                          in_=ap_of(src, 1, [[3, MP], [1, 3]]))

        b1d = ach.tile([128, MP, 6], F32)
        n0od = ach.tile([128, MP, 6], F32)
        ssm = ach.tile([128, MP], F32)
        tm3 = ach.tile([128, MP, 3], F32)
        v.tensor_sub(out=tm3[:], in0=ap_of(Ud, 12, [[18, MP], [1, 3]]),
                     in1=ap_of(Ud, 18, [[18, MP], [1, 3]]))
        tsq = ach.tile([128, MP, 3], F32)
        v.tensor_mul(out=tsq[:], in0=tm3[:], in1=tm3[:])
        v.tensor_reduce(out=ssm[:], in_=tsq[:], axis=AX.X, op=OP.add)
        v.tensor_scalar_max(out=ssm[:], in0=ssm[:], scalar1=1e-24)
        sc.activation(out=ssm[:], in_=ssm[:], func=AF.Sqrt)
        v.reciprocal(out=ssm[:], in_=ssm[:])
        v.tensor_tensor(out=ap_of(b1d, 0, [[6, MP], [1, 3]]), in0=tm3[:],
                        in1=ap_of(ssm, 0, [[1, MP], [0, 3]]), op=OP.mult)
        v.tensor_copy(out=ap_of(b1d, 3, [[6, MP], [1, 3]]),
                      in_=ap_of(b1d, 0, [[6, MP], [1, 3]]))
        v.tensor_copy(out=ap_of(n0od, 0, [[6, MP], [1, 3]]),
                      in_=ap_of(n0d, 12, [[18, MP], [1, 3]]))
        v.tensor_copy(out=ap_of(n0od, 3, [[6, MP], [1, 3]]),
                      in_=ap_of(n0od, 0, [[6, MP], [1, 3]]))
        v.tensor_copy(out=ap_of(CT, 21, [[32, MP], [1, 3]]),
                      in_=ap_of(b1d, 0, [[6, MP], [1, 3]]))
        v.tensor_copy(out=ap_of(CT, 24, [[32, MP], [1, 3]]),
                      in_=ap_of(n0od, 0, [[6, MP], [1, 3]]))
        v.tensor_tensor(out=tm3[:], in0=ap_of(b1d, 1, [[6, MP], [1, 3]]),
                        in1=ap_of(n0od, 2, [[6, MP], [1, 3]]), op=OP.mult)
        tm3b = ach.tile([128, MP, 3], F32)
        v.tensor_tensor(out=tm3b[:], in0=ap_of(b1d, 2, [[6, MP], [1, 3]]),
                        in1=ap_of(n0od, 1, [[6, MP], [1, 3]]), op=OP.mult)
        v.tensor_sub(out=ap_of(CT, 27, [[32, MP], [1, 3]]), in0=tm3[:], in1=tm3b[:])
        # fixups for chain-padding features + last node's Q, via tiny DMAs
        # (DVE can't address partition bases like 127)
        zz9 = cons.tile([1, 9], F32)
        v.memset(zz9[:], 0.0)
        one9 = cons.tile([1, 9], F32)
        v.memset(one9[:], 1.0)
        for col, srct in ((0, one9), (3, zz9), (6, one9), (9, zz9)):
            nc.sync.dma_start(out=ap_of(CT, col, [[1, 1]], p0=0, pcnt=1),
                              in_=ap_of(srct, 0, [[1, 1]], pcnt=1))
        for col, srct in ((1, one9), (4, zz9), (7, one9), (10, zz9)):
            nc.sync.dma_start(out=ap_of(CT, (MP - 1) * 32 + col, [[1, 2]], p0=127, pcnt=1),
                              in_=ap_of(srct, 0, [[1, 2]], pcnt=1))
        nc.sync.dma_start(out=ap_of(CT, (MP - 1) * 32 + 21, [[1, 9]], p0=127, pcnt=1),
                          in_=ap_of(zz9, 0, [[1, 9]], pcnt=1))

        AT = ach.tile([128, MP, 12], F32)
        nc.sync.dma_start(out=AT[:], in_=dap(xatoms, 0, [[12 * MP, 128], [1, 12 * MP]]))
        dXi = ach.tile([128, MP, 2, 3], F32)
        v.tensor_sub(out=dXi[:], in0=ap_of(AT, 6, [[12, MP], [3, 2], [1, 3]]),
                     in1=ap_of(AT, 0, [[12, MP], [0, 2], [1, 3]]))
        YT = ach.tile([128, MP, 2, 3], F32)
        tmq = ach.tile([128, MP, 2, 3], F32)
        for i in range(3):
            v.tensor_tensor(out=tmq[:], in0=dXi[:],
                            in1=ap_of(CT, 21 + 3 * i, [[32, MP], [0, 2], [1, 3]]),
                            op=OP.mult)
            v.tensor_reduce(out=ap_of(YT, i, [[6, MP], [3, 2]]), in_=tmq[:],
                            axis=AX.X, op=OP.add)
        v.tensor_mul(out=tmq[:], in0=YT[:], in1=YT[:])
        ssv = ach.tile([128, MP, 2], F32)
        v.tensor_reduce(out=ssv[:], in_=tmq[:], axis=AX.X, op=OP.add)
        v.tensor_scalar_max(out=ssv[:], in0=ssv[:], scalar1=1e-24)
        sc.activation(out=ssv[:], in_=ssv[:], func=AF.Sqrt)
        v.reciprocal(out=ssv[:], in_=ssv[:])
        v.tensor_tensor(out=ap_of(CT, 15, [[32, MP], [3, 2], [1, 3]]), in0=YT[:],
                        in1=ap_of(ssv, 0, [[2, MP], [1, 2], [0, 3]]), op=OP.mult)

        nodeT = ach.tile([32, n_sample], F32)
        for m in range(MP):
            pm = pss()
            pe.transpose(out=pm[0:32, :], in_=CT[:, m, :], identity=ident[:])
            v.tensor_copy(out=ap_of(nodeT, m, [[MP, 128]]), in_=pm[0:32, :])

        hvT = cons.tile([H, n_sample], F32)
        for c0 in range(0, n_sample, 512):
            c1 = min(c0 + 512, n_sample)
            ph = ps5.tile([H, 1024], F32, tag="big")
            pe.matmul(out=ph[:, 0:c1 - c0], lhsT=wv_sb[:],
                      rhs=nodeT[0:NODE_IN, c0:c1], start=True, stop=True)
            sc.activation(out=hvT[:, c0:c1], in_=ph[:, 0:c1 - c0],
                          func=AF.Identity, bias=wvb_t[:])
        stage32 = ach.tile([128, MP, H], F32)
        stageb = ach.tile([128, MP, H], BF16)
        for t in range(n_sample // 128):
            pb = pss()
            pe.transpose(out=pb[:], in_=hvT[:, 128 * t:128 * (t + 1)], identity=ident[:])
            v.tensor_copy(out=stage32[:, t, :], in_=pb[:])
            v.tensor_copy(out=stageb[:, t, :], in_=pb[:])
        nc.sync.dma_start(
            out=dap(hvf_table, 0, [[H, 128], [128 * H, MP], [1, H]]), in_=stage32[:])
        nc.sync.dma_start(
            out=dap(hv_table, 0, [[H, 128], [128 * H, MP], [1, H]]), in_=stageb[:])
        zz = ach.tile([128, GROW], F32)
        v.memset(zz[:], 0.0)
        nc.sync.dma_start(
            out=dap(geom_table, 0, [[GROW * 128, MP], [GROW, 128], [1, GROW]]),
            in_=ap_of(zz, 0, [[0, MP], [1, GROW]]))
        nc.sync.dma_start(
            out=dap(geom_table, 0, [[GROW * MP, 128], [GROW, MP], [1, 9]]),
            in_=ap_of(CT, 21, [[32, MP], [1, 9]]))
        nc.sync.dma_start(
            out=dap(geom_table, 9, [[GROW * MP, 128], [GROW, MP], [1, 3]]),
            in_=ap_of(AT, 3, [[12, MP], [1, 3]]))
        nc.sync.dma_start(
            out=dap(geom_table, 12, [[GROW * MP, 128], [GROW, MP], [1, 3]]),
            in_=ap_of(AT, 0, [[12, MP], [1, 3]]))
        nc.sync.dma_start(
            out=dap(geom_table, 15, [[GROW * MP, 128], [GROW, MP], [1, 6]]),
            in_=ap_of(AT, 6, [[12, MP], [1, 6]]))

        # (no barrier: Tile tracks DRAM table-write -> gather deps via APs)

        idx_sb = cons.tile([128, ICOL], mybir.dt.int16)
        nc.sync.dma_start(out=idx_sb[:], in_=idx16r.ap())

        # ---------- phase B: edge blocks ----------
        PHB = int(os.environ.get('KPHB', '9'))
        for bl in range(BLK if PHB >= 1 else 0):
            hvnbrT = hvn.tile([128, 1, 30 * 128], BF16, tag="hvn")
            gs.dma_gather(out_ap=hvnbrT[:], in_ap=hv_table.ap(),
                          idxs_ap=idx_sb[:, 248 * bl:248 * bl + 240],
                          num_idxs=3840, num_idxs_reg=3840, elem_size=H,
                          transpose=True, single_packet=False)
            G = gp.tile([128, 31, GROW], F32, tag="G")
            gs.dma_gather(out_ap=G[:], in_ap=geom_table.ap(),
                          idxs_ap=idx_sb[:, 248 * bl:248 * (bl + 1)],
                          num_idxs=3968, num_idxs_reg=3968, elem_size=GROW,
                          transpose=False, single_packet=False)
            sc0 = 248 * bl + 240
            hvselfN = trk.tile([128, 1, H], F32, tag="hvself")
            gs.dma_gather(out_ap=hvselfN[:], in_ap=hvf_table.ap(),
                          idxs_ap=idx_sb[:, sc0:sc0 + 8],
                          num_idxs=128, num_idxs_reg=128, elem_size=H,
                          transpose=False)

            if PHB < 2:
                continue
            EQ = eqp.tile([128, 32, 32], F32, tag="EQ")
            v.memset(ap_of(EQ, 16, [[32, 32], [1, 16]]), 0.0)
            v.memset(ap_of(EQ, 30 * 32, [[1, 64]]), 0.0)
            dXn = eqp.tile([128, 30, 4, 3], F32, tag="dXn")
            v.tensor_sub(out=dXn[:], in0=ap_of(G, 9, [[GROW, 30], [3, 4], [1, 3]]),
                         in1=ap_of(G, 30 * GROW + 12, [[0, 30], [0, 4], [1, 3]]))
            Y = eqp.tile([128, 3, 30, 4], F32, tag="Y")
            tg = eqp.tile([128, 30, 4, 3], F32, tag="tg")
            for i in range(3):
                v.tensor_tensor(out=tg[:], in0=dXn[:],
                                in1=ap_of(G, 30 * GROW + 3 * i, [[0, 30], [0, 4], [1, 3]]),
                                op=OP.mult)
                v.tensor_reduce(out=Y[:, i, :, :], in_=tg[:], axis=AX.X, op=OP.add)
            sqY = eqp.tile([128, 3, 30, 4], F32, tag="sqY")
            v.tensor_mul(out=sqY[:], in0=Y[:], in1=Y[:])
            ssE = eqp.tile([128, 30, 4], F32, tag="ssE")
            v.tensor_reduce(out=ssE[:], in_=ap_of(sqY, 0, [[4, 30], [1, 4], [120, 3]]),
                            axis=AX.X, op=OP.add)
            v.tensor_scalar_max(out=ssE[:], in0=ssE[:], scalar1=1e-24)
            sc.activation(out=ssE[:], in_=ssE[:], func=AF.Sqrt)
            v.reciprocal(out=ssE[:], in_=ssE[:])
            v.tensor_tensor(out=ap_of(EQ, 0, [[32, 30], [3, 4], [1, 3]]),
                            in0=ap_of(Y, 0, [[4, 30], [1, 4], [120, 3]]),
                            in1=ap_of(ssE, 0, [[4, 30], [1, 4], [0, 3]]), op=OP.mult)
            RT = eqp.tile([128, 3, 30, 3], F32, tag="RT")
            tR = eqp.tile([128, 30, 3, 3], F32, tag="tR")
            for i in range(3):
                v.tensor_tensor(out=tR[:], in0=ap_of(G, 0, [[GROW, 30], [1, 3], [3, 3]]),
                                in1=ap_of(G, 30 * GROW + i, [[0, 30], [0, 3], [3, 3]]),
                                op=OP.mult)
                v.tensor_reduce(out=RT[:, i, :, :], in_=tR[:], axis=AX.X, op=OP.add)
            s_ = trk.tile([128, 30], F32, tag="s_")
            v.tensor_add(out=s_[:], in0=ap_of(RT, 0, [[3, 30]]),
                         in1=ap_of(RT, 90 + 1, [[3, 30]]))
            v.tensor_add(out=s_[:], in0=s_[:], in1=ap_of(RT, 180 + 2, [[3, 30]]))
            dgm = trk.tile([128, 3, 30], F32, tag="dgm")
            for i in range(3):
                v.tensor_copy(out=dgm[:, i, :], in_=ap_of(RT, 90 * i + i, [[3, 30]]))
            mm_ = trk.tile([128, 3, 30], F32, tag="mm_")
            v.scalar_tensor_tensor(out=mm_[:], in0=dgm[:], scalar=2.0,
                                   in1=ap_of(s_, 0, [[0, 3], [1, 30]]),
                                   op0=OP.mult, op1=OP.subtract)
            mg = trk.tile([128, 3, 30], F32, tag="mg")
            sc.activation(out=mg[:], in_=mm_[:], func=AF.Abs, bias=c_one[:])
            sc.activation(out=mg[:], in_=mg[:], func=AF.Sqrt)
            dd = trk.tile([128, 3, 30], F32, tag="dd")
            for m_, (ia, ib) in enumerate((((2, 1), (1, 2)), ((0, 2), (2, 0)),
                                           ((1, 0), (0, 1)))):
                v.tensor_sub(out=dd[:, m_, :],
                             in0=ap_of(RT, 90 * ia[0] + ia[1], [[3, 30]]),
                             in1=ap_of(RT, 90 * ib[0] + ib[1], [[3, 30]]))
            sg_ = trk.tile([128, 3, 30], F32, tag="sg_")
            sc.activation(out=sg_[:], in_=dd[:], func=AF.Sign)
            xyz = trk.tile([128, 3, 30], F32, tag="xyz")
            v.tensor_mul(out=xyz[:], in0=sg_[:], in1=mg[:])
            w_ = trk.tile([128, 30], F32, tag="w_")
            sc.activation(out=w_[:], in_=s_[:], func=AF.Relu, bias=c_one[:])
            sc.activation(out=w_[:], in_=w_[:], func=AF.Sqrt)
            qss = trk.tile([128, 30], F32, tag="qss")
            sqx = trk.tile([128, 3, 30], F32, tag="sqx")
            v.tensor_mul(out=sqx[:], in0=xyz[:], in1=xyz[:])
            v.tensor_reduce(out=qss[:], in_=ap_of(sqx, 0, [[1, 30], [30, 3]]),
                            axis=AX.X, op=OP.add)
            tw = trk.tile([128, 30], F32, tag="tw")
            v.tensor_mul(out=tw[:], in0=w_[:], in1=w_[:])
            v.tensor_add(out=qss[:], in0=qss[:], in1=tw[:])
            v.tensor_scalar_max(out=qss[:], in0=qss[:], scalar1=1e-24)
            sc.activation(out=qss[:], in_=qss[:], func=AF.Sqrt)
            v.reciprocal(out=qss[:], in_=qss[:])
            v.tensor_tensor(out=ap_of(EQ, 12, [[32, 30], [1, 3]]),
                            in0=ap_of(xyz, 0, [[1, 30], [30, 3]]),
                            in1=ap_of(qss, 0, [[1, 30], [0, 3]]), op=OP.mult)
            v.tensor_tensor(out=ap_of(EQ, 15, [[32, 30]]), in0=w_[:], in1=qss[:],
                            op=OP.mult)

            EQT = eqp.tile([128, 8, 128], F32, tag="EQT")
            for g in range(8):
                pg = pss()
                pe.matmul(out=pg[:], lhsT=ap_of(EQ, g * 128, [[1, 128]]),
                          rhs=ident[:], is_transpose=True, start=True, stop=True)
                v.tensor_copy(out=EQT[:, g, :], in_=pg[:])

            if PHB < 3:
                continue
            # self hv transposed for the W1a term
            pselfT = pss()
            pe.transpose(out=pselfT[:], in_=hvselfN[:, 0, :], identity=ident[:])
            hvselfT = trk.tile([128, 128], F32, tag="hvselfT")
            v.tensor_copy(out=hvselfT[:], in_=pselfT[:])

            m2_sb = m2p.tile([128, 30, 128], BF16, tag="m2")
            for ch in range(4):
                kk = 8 * ch
                nk = min(8, 30 - kk)
                ncol = nk * 128
                ps1 = ps5.tile([128, 1024], F32, tag="big")
                for j in range(nk):
                    kcur = kk + j
                    t_i, jj = divmod(kcur, 4)
                    r0 = 128 * j
                    pe.matmul(out=ps1[:, r0:r0 + 128], lhsT=w1a_sb[:],
                              rhs=hvselfT[:], start=True, stop=False)
                    pe.matmul(out=ps1[:, r0:r0 + 128], lhsT=w1cb[:],
                              rhs=hvnbrT[:, 0, 128 * kcur:128 * kcur + 128],
                              start=False, stop=False)
                    pe.matmul(out=ps1[:, r0:r0 + 128],
                              lhsT=wbf128[32 * jj:32 * jj + 32, :],
                              rhs=EQT[32 * jj:32 * jj + 32, t_i, :],
                              start=False, stop=True, tile_position=(32 * jj, 0))
                m1_sb = m1p.tile([128, 1024], BF16, tag="m1")
                sc.activation(out=m1_sb[:, 0:ncol], in_=ps1[:, 0:ncol], func=AF.Gelu,
                              bias=bias1[:])
                ps2 = ps5.tile([128, 1024], F32, tag="big")
                for half in range((ncol + 511) // 512):
                    c0 = 512 * half
                    c1 = min(c0 + 512, ncol)
                    pe.matmul(out=ps2[:, c0:c1], lhsT=w2b[:], rhs=m1_sb[:, c0:c1],
                              start=True, stop=True)
                sc.activation(out=ap_of(m2_sb, kk * 128, [[1, ncol]]),
                              in_=ps2[:, 0:ncol], func=AF.Gelu, bias=w2b_t[:])

            if PHB < 4:
                continue
            ps_dh = psD.tile([128, 128], F32, tag="dh")
            for k_ in range(30):
                pe.matmul(out=ps_dh[:], lhsT=w3p[:], rhs=m2_sb[:, k_, :],
                          start=(k_ == 0), stop=(k_ == 29))

            # trunk in node-layout (PE transpose reads SBUF only)
            dh_ct = trk.tile([128, 128], F32, tag="dh_ct")
            v.tensor_copy(out=dh_ct[:], in_=ps_dh[:])
            pdn2 = pss()
            pe.transpose(out=pdn2[:], in_=dh_ct[:], identity=ident[:])
            t_n = trk.tile([128, 128], F32, tag="t_n")
            v.tensor_copy(out=t_n[:], in_=pdn2[:])
            v.tensor_add(out=t_n[:], in0=t_n[:], in1=lnbc[:, 4, :])
            v.tensor_add(out=t_n[:], in0=t_n[:], in1=hvselfN[:, 0, :])

            def lnorm(xn, gcol, bcol, otile):
                st6 = trk.tile([128, 6], F32, tag="st6")
                v.bn_stats(out=st6[:], in_=xn[:])
                mv = trk.tile([128, 2], F32, tag="mv")
                v.bn_aggr(out=mv[:], in_=st6[:])
                rs = trk.tile([128, 1], F32, tag="rs")
                sc.activation(out=rs[:], in_=mv[:, 1:2], func=AF.Sqrt, bias=c_eps[:])
                v.reciprocal(out=rs[:], in_=rs[:])
                v.tensor_scalar(out=otile[:], in0=xn[:], scalar1=mv[:, 0:1],
                                scalar2=rs[:], op0=OP.subtract, op1=OP.mult)
                v.tensor_mul(out=otile[:], in0=otile[:], in1=lnbc[:, gcol, :])
                v.tensor_add(out=otile[:], in0=otile[:], in1=lnbc[:, bcol, :])
                return otile

            hv1n = trk.tile([128, 128], F32, tag="hv1n")
            lnorm(t_n, 0, 1, hv1n)
            ph1 = pss()
            pe.transpose(out=ph1[:], in_=hv1n[:], identity=ident[:])
            hv1T = trk.tile([128, 128], F32, tag="hv1T")
            v.tensor_copy(out=hv1T[:], in_=ph1[:])
            ps_fo = psD.tile([128, 128], F32, tag="fo")
            for blkf in range(4):
                pf = pss()
                pe.matmul(out=pf[:], lhsT=wi_sb[:, 128 * blkf:128 * (blkf + 1)],
                          rhs=hv1T[:], start=True, stop=True)
                f_sb = trk.tile([128, 128], BF16, tag="f_sb")
                sc.activation(out=f_sb[:], in_=pf[:], func=AF.Gelu,
                              bias=wib_t[:, blkf:blkf + 1])
                pe.matmul(out=ps_fo[:], lhsT=wo_b16[:, blkf, :], rhs=f_sb[:],
                          start=(blkf == 0), stop=(blkf == 3))
            fo_ct = trk.tile([128, 128], F32, tag="fo_ct")
            sc.activation(out=fo_ct[:], in_=ps_fo[:], func=AF.Identity, bias=wob_t[:])
            v.tensor_add(out=fo_ct[:], in0=fo_ct[:], in1=hv1T[:])
            pt2 = pss()
            pe.transpose(out=pt2[:], in_=fo_ct[:], identity=ident[:])
            t2n = trk.tile([128, 128], F32, tag="t2n")
            v.tensor_copy(out=t2n[:], in_=pt2[:])
            on = trk.tile([128, 128], F32, tag="on")
            lnorm(t2n, 2, 3, on)
            nc.sync.dma_start(out=dap(out, 128 * bl * H, [[H, 128], [1, H]]),
                              in_=on[:])

        PHB2 = int(os.environ.get('KPHB', '9'))
        if PHB2 < 9:
            for bl in range(BLK):
                onz = trk.tile([128, 128], F32, tag="onz")
                v.memset(onz[:], 0.5)
                nc.sync.dma_start(out=dap(out, 128 * bl * H, [[H, 128], [1, H]]),
                                  in_=onz[:])

    nc.compile()
    return nc


_CACHE = {}


def get_module(n_sample=N, node_cnt=NPC, num_devices=8):
    key = (n_sample, node_cnt, num_devices)
    if key not in _CACHE:
        _CACHE[key] = build_kernel(n_sample, node_cnt, num_devices)
    return _CACHE[key]


def make_core_inputs(inputs, s, lo, cnt, n_sample):
    """Host-side per-core marshalling (layout only, no float math)."""
    X = np.asarray(inputs['X'][s], np.float32)
    Xf = X[:, :3, :].reshape(3 * n_sample, 3)
    xp_ = np.zeros((3 * n_sample + 4, 3), np.float32)
    xp_[2:2 + 3 * n_sample] = Xf
    xatoms_ = np.ascontiguousarray(X.reshape(n_sample, 12))
    E = np.asarray(inputs['E_idx'][s])
    blocks = cnt // 128
    flat = np.empty(blocks * 3968, np.int64)
    for bl in range(blocks):
        blk = E[lo + 128 * bl: lo + 128 * (bl + 1)]        # [128,K]
        flat[bl * 3968:bl * 3968 + 3840] = blk.T.reshape(-1)
        flat[bl * 3968 + 3840:(bl + 1) * 3968] = np.arange(
            lo + 128 * bl, lo + 128 * (bl + 1))
    ncol = flat.size // 16
    idx = np.empty((16, ncol), np.int16)
    for i in range(16):
        idx[i, :] = flat[i::16].astype(np.int16)
    idx16r_ = np.tile(idx, (8, 1))
    d = {'xp': xp_, 'xatoms': xatoms_, 'idx16r': idx16r_}
    remap = {'wv_w': 'Wv_w', 'wv_b': 'Wv_b', 'we_w': 'We_w', 'we_b': 'We_b',
             'w1': 'W1_w', 'w1_b': 'W1_b', 'w2': 'W2_w', 'w2_b': 'W2_b',
             'w3': 'W3_w', 'w3_b': 'W3_b', 'wi': 'Wi_w', 'wi_b': 'Wi_b',
             'wo': 'Wo_w', 'wo_b': 'Wo_b', 'ln1_g': 'ln1_g', 'ln1_b': 'ln1_b',
             'ln2_g': 'ln2_g', 'ln2_b': 'ln2_b'}
    for k_, v_ in remap.items():
        d[k_] = np.ascontiguousarray(np.asarray(inputs[v_], np.float32))
    return d


def kernel(**inputs):
    n_cores = 8
    nc = get_module(N, NPC, n_cores)
    in_maps = []
    for c in range(n_cores):
        s, h = divmod(c, 2)
        in_maps.append(make_core_inputs(inputs, s, h * NPC, NPC, N))
    res = run_bass_kernel_spmd(nc, in_maps, core_ids=list(range(n_cores)))
    out = np.empty((B, N, H), np.float32)
    for c in range(n_cores):
        s, h = divmod(c, 2)
        out[s, h * NPC:(h + 1) * NPC] = res.results[c]["out"]
    return out
